# revision 1
# baseline (speedup 1.0000x reference)
"""DeepseekV3 MoE layer on 8 Trainium2 NeuronCores.

Strategy (expert-parallel, per sharding hint):
- Each core owns 2 of the 16 routed experts. The host routes tokens by top-4
  gate scores (fp32, identical to reference) and ships each core its experts'
  gathered tokens pre-transposed, plus the normalized combine weights
  (host-side gate math, same class of work as the top-k routing).
- All device inputs are shipped in their exact SBUF layouts so every preload
  DMA is contiguous per partition (8-16KB lines) and balanced across the
  three DMA queues (sync / scalar / gpsimd).
- The device runs the SwiGLU expert MLPs fp16 (fp32 PSUM), scales outputs by
  the combine weights into per-expert staging buffers, and scatter-adds them
  into a per-core partial-output y_acc in DRAM with two batched indirect
  DMAs per expert (low/high token halves) to keep the gpsimd engine free.
- The shared expert is sharded along its intermediate dim (128 of 1024 per
  core), computed weight-stationary so its intermediate lands pre-transposed
  ([i, t]); its dense per-tile output initializes y_acc.
- y_acc is reduce-scattered in NHALF token chunks, each fired as soon as the
  scatters touching that chunk complete, so all but the last chunk overlap
  compute. The host reassembles the fp16 outputs and casts (pure unshard).
"""

import os
import sys
import types

sys.path.insert(0, "/opt/trn_rl_repo")

# antenv.axon_hooks shim so trace=True works under axon (profiling only).
if "antenv.axon_hooks" not in sys.modules:
    _hook_holder = [None]
    _hooks_mod = types.ModuleType("antenv.axon_hooks")
    _hooks_mod.set_axon_ntff_profile_hook = lambda h: _hook_holder.__setitem__(0, h)
    _hooks_mod.get_axon_ntff_profile_hook = lambda: _hook_holder[0]
    sys.modules["antenv.axon_hooks"] = _hooks_mod
    try:
        from trn_agent_boot.trn_boot import _ntff_profile_via_ctypes

        _hook_holder[0] = _ntff_profile_via_ctypes("/opt/axon/libaxon_pjrt.so")
    except Exception:
        pass

import ml_dtypes
import numpy as np

import concourse.bass as bass
import concourse.mybir as mybir
from concourse import bacc
from concourse.tile import TileContext, add_dep_helper
from concourse.bass_utils import run_bass_kernel_spmd

N_CORES = 8
T, H, E, I = 2048, 1024, 16, 512
TOPK = 4
SIC = 128  # shared-expert intermediate slice per core (1024 / 8)
EPC = 2  # experts per core
OOB = 1 << 20
NHALF = int(os.environ.get('KERNEL_NHALF', '1'))  # reduce-scatter chunks
TH = T // NHALF
TQ = 4  # dense-write granularity (512-token tiles)
QT = T // TQ

F16 = mybir.dt.float16
F8 = mybir.dt.float8e4
F32 = mybir.dt.float32
I32 = mybir.dt.int32
AF = mybir.ActivationFunctionType

_nc_cache = {}
last_exec_time_ns = None


def _build(C_use, C_pad, batches, edges, touch_tq, touch_q):
    """batches: tuple of (e, c0, c1) scatter batches in emission order.
    edges: chunk-level (i0, jj) pairs where expert-0 chunk i0 and expert-1
    chunk jj may touch the same y_acc rows (RMW adds must serialize).
    touch_tq[bi] / touch_q[h] are keyed by batch index."""
    NCC = C_pad // 128
    nc = bacc.Bacc(trn_type="TRN2", target_bir_lowering=False, num_devices=N_CORES)

    # ---- I/O (all pre-arranged to SBUF layout on host; contiguous DMAs) ----
    xT16 = nc.dram_tensor("xT16", [TQ, 128, H // 128, QT], F16, kind="ExternalInput")
    xgT16 = nc.dram_tensor("xgT16", [EPC, 128, H // 128, C_pad], F16, kind="ExternalInput")
    wgu16 = nc.dram_tensor("wgu16", [EPC, 2, 128, 2, H // 128, I // 2], F16, kind="ExternalInput")
    wd16 = nc.dram_tensor("wd16", [EPC, 128, 2, 2, H], F8, kind="ExternalInput")
    sgsu16 = nc.dram_tensor("sgsu16", [128, H // 128, 2 * SIC], F16, kind="ExternalInput")
    sd16 = nc.dram_tensor("sd16", [SIC, H], F16, kind="ExternalInput")
    sidx = nc.dram_tensor("sidx", [128, EPC * NCC], I32, kind="ExternalInput")
    wG = nc.dram_tensor("wG", [128, EPC * NCC], F32, kind="ExternalInput")

    y_acc = nc.dram_tensor("y_acc", [T, H], F16)
    rows = TH // N_CORES
    rs_b = nc.dram_tensor("rs_b", [NHALF * rows, H], F16)
    y_out = nc.dram_tensor("y_out", [NHALF * rows, H], F16, kind="ExternalOutput")

    SS = 2 * SIC  # 256

    with TileContext(nc) as tc:
        with (
            tc.tile_pool(name="res", bufs=1) as res,
            tc.tile_pool(name="sc", bufs=4) as scp,
            tc.tile_pool(name="ds", bufs=2) as dsp,
            tc.tile_pool(name="ps_a", bufs=4, space="PSUM") as ps_a,
            tc.tile_pool(name="ps_gu", bufs=2, space="PSUM") as ps_gu,
        ):
            # ---- resident tiles ----
            xT_sb = [res.tile([128, H // 128, QT], F16, tag=f"xT{q}",
                              name=f"xT_sb{q}") for q in range(TQ)]
            xgT_sb = res.tile([128, EPC, H // 128, C_pad], F16, tag="xgT")
            wgu_sb = res.tile([128, EPC, 2, 2, H // 128, I // 2], F16, tag="wgu")
            wd_sb = res.tile([128, EPC, 2, 2, H], F8, tag="wd")
            sgsu_sb = res.tile([128, H // 128, SS], F16, tag="sgsu")
            sd_sb = res.tile([128, H], F16, tag="sd")
            sidx_sb = res.tile([128, EPC * NCC], I32, tag="sidx")
            wG_sb = res.tile([128, EPC * NCC], F32, tag="wG")
            p_sb = res.tile([128, EPC, 2, 2, C_pad], F8, tag="p")
            sp_sb = res.tile([128, T], F16, tag="sp")
            yg_sb = [res.tile([128, NCC, H], F16, tag=f"yg{e}",
                              name=f"yg_sb{e}") for e in range(EPC)]

            # ---- preload: balanced across the three DMA queues. The tile
            # scheduler reorders independent DMAs, so each queue's order is
            # enforced with an explicit dependency chain (first-use first). ----
            nc.sync.dma_start(xT_sb[0][:], xT16.ap()[0])
            nc.sync.dma_start(xT_sb[1][:], xT16.ap()[1])
            nc.sync.dma_start(sd_sb[:], sd16.ap())
            nc.sync.dma_start(xT_sb[2][:], xT16.ap()[2])
            nc.sync.dma_start(xT_sb[3][:], xT16.ap()[3])
            nc.sync.dma_start(sidx_sb[:], sidx.ap())
            nc.sync.dma_start(wG_sb[:], wG.ap())
            nc.scalar.dma_start(sgsu_sb[:], sgsu16.ap())
            nc.scalar.dma_start(wgu_sb[:, 0, 0], wgu16.ap()[0, 0])
            nc.scalar.dma_start(wgu_sb[:, 0, 1], wgu16.ap()[0, 1])
            nc.scalar.dma_start(wd_sb[:, 0], wd16.ap()[0])
            nc.scalar.dma_start(wd_sb[:, 1], wd16.ap()[1])
            nc.gpsimd.dma_start(xgT_sb[:, 0], xgT16.ap()[0])
            nc.gpsimd.dma_start(wgu_sb[:, 1, 0], wgu16.ap()[1, 0])
            nc.gpsimd.dma_start(wgu_sb[:, 1, 1], wgu16.ap()[1, 1])
            nc.gpsimd.dma_start(xgT_sb[:, 1], xgT16.ap()[1])

            # zero the pad columns of p (read by down-matmul lhsT chunks)
            if C_pad > C_use:
                nc.vector.memset(p_sb[:, :, :, :, C_use:C_pad], 0)

            # gate/up token blocks of 256 (ldweights still overlaps matmuls;
            # small blocks let downs/scatters/RS start early)
            segs = []
            s0 = 0
            while s0 < C_use:
                s1 = min(s0 + 256, C_use)
                segs.append((s0, s1))
                s0 = s1

            # ---- shared expert: weight-stationary gate/up; sp lands
            # pre-transposed [i, t] so the down matmul needs no transposes ----
            def emit_su(tq):
                ps_ic = []
                for ic in range(2):
                    psu = ps_a.tile([128, QT], F32, tag="psa")
                    for ho in range(H // 128):
                        nc.tensor.matmul(
                            psu[:],
                            lhsT=sgsu_sb[:, ho, ic * 128:(ic + 1) * 128],
                            rhs=xT_sb[tq][:, ho, :],
                            start=(ho == 0),
                            stop=(ho == H // 128 - 1),
                        )
                    ps_ic.append(psu)
                sg_t = scp.tile([128, QT], F16, tag="sg")
                nc.scalar.activation(sg_t[:], ps_ic[0][:], AF.Silu)
                nc.vector.tensor_tensor(
                    out=sp_sb[:, tq * QT:(tq + 1) * QT], in0=sg_t[:], in1=ps_ic[1][:],
                    op=mybir.AluOpType.mult,
                )

            dense_wr = [None] * TQ

            def emit_dense(tq):
                ys = dsp.tile([128, 4, H], F16, tag="ys")
                for tc4 in range(4):
                    t0 = tq * QT + tc4 * 128
                    for hf in range(2):
                        pso = ps_a.tile([128, 512], F32, tag="psa")
                        nc.tensor.matmul(
                            pso[:],
                            lhsT=sp_sb[:, t0:t0 + 128],
                            rhs=sd_sb[:, hf * 512:(hf + 1) * 512],
                            start=True,
                            stop=True,
                        )
                        nc.vector.tensor_copy(
                            ys[:, tc4, hf * 512:(hf + 1) * 512], pso[:])
                dense_wr[tq] = nc.sync.dma_start(
                    y_acc.ap()[tq * QT:(tq + 1) * QT, :].rearrange(
                        "(tc p) h -> p tc h", p=128),
                    ys[:],
                )

            # ---- routed experts: g/u -> p = silu(g)*u for one token block ----
            def emit_gu(e, a, b):
                for it in range(I // 128):
                    pg_full = ps_gu.tile([128, 512], F32, tag="pg")
                    pg = pg_full[:, :b - a]
                    pu_full = ps_gu.tile([128, 512], F32, tag="pu")
                    pu = pu_full[:, :b - a]
                    for ho in range(H // 128):
                        nc.tensor.matmul(
                            pg[:],
                            lhsT=wgu_sb[:, e, it // 2, 0, ho,
                                        (it % 2) * 128:(it % 2) * 128 + 128],
                            rhs=xgT_sb[:, e, ho, a:b],
                            start=(ho == 0),
                            stop=(ho == H // 128 - 1),
                        )
                        nc.tensor.matmul(
                            pu[:],
                            lhsT=wgu_sb[:, e, it // 2, 1, ho,
                                        (it % 2) * 128:(it % 2) * 128 + 128],
                            rhs=xgT_sb[:, e, ho, a:b],
                            start=(ho == 0),
                            stop=(ho == H // 128 - 1),
                        )
                    sg2_full = scp.tile([128, 512], F16, tag="sg2")
                    sg2 = sg2_full[:, :b - a]
                    nc.scalar.activation(sg2[:], pg[:], AF.Silu)
                    nc.vector.tensor_tensor(
                        out=p_sb[:, e, it // 2, it % 2, a:b], in0=sg2[:],
                        in1=pu[:], op=mybir.AluOpType.mult,
                    )

            # ---- routed expert down matmul + combine-weight scale ----
            def emit_down(e, cc):
                j = e * NCC + cc
                for hf in range(2):
                    py = ps_a.tile([128, 512], F32, tag="psa")
                    for kp in range(2):
                        nc.tensor.matmul(
                            py[:],
                            lhsT=p_sb[:, e, kp, :, cc * 128:(cc + 1) * 128],
                            rhs=wd_sb[:, e, kp, :, hf * 512:(hf + 1) * 512],
                            start=(kp == 0),
                            stop=(kp == 1),
                            perf_mode=mybir.MatmulPerfMode.DoubleRow,
                        )
                    nc.vector.tensor_scalar_mul(
                        yg_sb[e][:, cc, hf * 512:(hf + 1) * 512],
                        py[:], wG_sb[:, j:j + 1])

            su_done = [False] * TQ

            def ensure_dense(tq):
                if dense_wr[tq] is None:
                    if not su_done[tq]:
                        emit_su(tq)
                        su_done[tq] = True
                    emit_dense(tq)

            scat_insts = {}  # (e, cc) -> inst
            batch_done = set()
            rs_insts = [None] * NHALF

            def emit_scatter(bi):
                e, c0, c1 = batches[bi]
                for tq in touch_tq.get(bi, ()):
                    ensure_dense(tq)
                for cc in range(c0, c1 + 1):
                    j = e * NCC + cc
                    sc = nc.gpsimd.indirect_dma_start(
                        out=y_acc[:],
                        out_offset=bass.IndirectOffsetOnAxis(
                            ap=sidx_sb[:, j:j + 1], axis=0),
                        in_=yg_sb[e][:, cc, :],
                        in_offset=None,
                        bounds_check=T - 1,
                        oob_is_err=False,
                        compute_op=mybir.AluOpType.add,
                    )
                    for tq in touch_tq.get(bi, ()):
                        add_dep_helper(sc.ins, dense_wr[tq].ins,
                                       reason="scatter after dense init")
                    for (i0, jj) in edges:
                        other = None
                        if e == 1 and jj == cc:
                            other = (0, i0)
                        elif e == 0 and i0 == cc:
                            other = (1, jj)
                        if other is not None and other in scat_insts:
                            add_dep_helper(sc.ins, scat_insts[other].ins,
                                           reason="serialize colliding scatters")
                    scat_insts[(e, cc)] = sc
                batch_done.add(bi)

            def maybe_rs():
                for h in range(NHALF):
                    if rs_insts[h] is None:
                        if all(bi in batch_done for bi in touch_q.get(h, ())):
                            for tq in range(TQ):
                                if (tq * QT < (h + 1) * TH) and ((tq + 1) * QT > h * TH):
                                    ensure_dense(tq)
                            cc_inst = nc.gpsimd.collective_compute(
                                "ReduceScatter",
                                mybir.AluOpType.add,
                                replica_groups=[list(range(N_CORES))],
                                ins=[y_acc.ap()[h * TH:(h + 1) * TH, :].opt()],
                                outs=[rs_b.ap()[h * rows:(h + 1) * rows, :].opt()],
                            )
                            for bi in touch_q.get(h, ()):
                                e, c0, c1 = batches[bi]
                                for cc in range(c0, c1 + 1):
                                    add_dep_helper(
                                        cc_inst.ins, scat_insts[(e, cc)].ins,
                                        reason="rs after scatters")
                            for tq in range(TQ):
                                if (tq * QT < (h + 1) * TH) and ((tq + 1) * QT > h * TH):
                                    add_dep_helper(cc_inst.ins, dense_wr[tq].ins,
                                                   reason="rs after dense init")
                            rs_insts[h] = cc_inst
                            out_wr = nc.sync.dma_start(
                                y_out.ap()[h * rows:(h + 1) * rows, :],
                                rs_b.ap()[h * rows:(h + 1) * rows, :],
                            )
                            add_dep_helper(out_wr.ins, cc_inst.ins,
                                           reason="copy rs out")
                        else:
                            break  # keep cc-stream order h ascending

            # ---- emission order: shared front tiles, then per-expert
            # low-half pipeline (so RS chunk 0 fires early), then high ----
            emit_su(0)
            su_done[0] = True
            emit_su(1)
            su_done[1] = True
            emit_dense(0)
            emit_dense(1)

            gu_blocks = [0, 0]

            def gu_through(e, cc):
                while gu_blocks[e] * 256 < (cc + 1) * 128:
                    a, b = segs[gu_blocks[e]]
                    emit_gu(e, a, b)
                    gu_blocks[e] += 1

            for bi, (e, c0, c1) in enumerate(batches):
                gu_through(e, c1)
                for cc in range(c0, c1 + 1):
                    emit_down(e, cc)
                emit_scatter(bi)
                maybe_rs()
            for tq in range(TQ):
                ensure_dense(tq)
            maybe_rs()
            assert all(r is not None for r in rs_insts)

    nc.compile()
    return nc


def kernel(hidden_states, gate_w, expert_gate, expert_up, expert_down,
           shared_gate, shared_up, shared_down):
    global last_exec_time_ns
    B, S, Hh = hidden_states.shape
    x = np.asarray(hidden_states, np.float32).reshape(-1, Hh)

    # ---- host-side routing (the all-to-all dispatch, done as sharding) ----
    gw = np.asarray(gate_w, np.float32)
    scores = 1.0 / (1.0 + np.exp(-(x @ gw.T)))
    order = np.argsort(-scores, axis=1, kind="stable")[:, :TOPK]
    topk_w = np.take_along_axis(scores, order, axis=1)
    topk_w = topk_w / (topk_w.sum(-1, keepdims=True) + 1e-20)
    comb = np.zeros((T, E), np.float32)
    np.add.at(comb, (np.arange(T)[:, None], order), topk_w)

    sel = np.zeros((T, E), dtype=bool)
    sel[np.arange(T)[:, None], order] = True
    counts = sel.sum(0)
    C_use = int(max(64, -(-int(counts.max()) // 64) * 64))
    C_use = min(C_use, T)
    C_pad = -(-C_use // 128) * 128
    NCC = C_pad // 128

    gidx_all = np.zeros((E, C_pad), np.int32)
    sidx_all = np.full((E, C_pad), OOB, np.int32)
    for e in range(E):
        lst = np.nonzero(sel[:, e])[0].astype(np.int32)
        gidx_all[e, :len(lst)] = lst
        sidx_all[e, :len(lst)] = lst

    # ---- scatter batches: one per (expert, chunk), interleaved across the
    # two local experts in token order so the low RS chunk fires early ----
    batches = tuple((k, cc, cc) for cc in range(NCC) for k in range(EPC))

    # ---- cast / pack per-core inputs in exact SBUF layouts ----
    x16 = x.astype(np.float16)
    xT4 = np.ascontiguousarray(
        x16.T.reshape(H // 128, 128, TQ, QT).transpose(2, 1, 0, 3))
    eg = np.asarray(expert_gate, np.float32).astype(np.float16)
    eu = (np.asarray(expert_up, np.float32) * 8.0).astype(np.float16)
    ed = np.asarray(expert_down, np.float32)
    sg = np.asarray(shared_gate, np.float32).astype(np.float16)
    su = np.asarray(shared_up, np.float32).astype(np.float16)
    sd = np.asarray(shared_down, np.float32).astype(np.float16)

    in_maps = []
    for c in range(N_CORES):
        ex = [EPC * c + k for k in range(EPC)]
        xgT = np.stack([
            np.ascontiguousarray(
                x16[gidx_all[e]].T.reshape(H // 128, 128, C_pad).transpose(1, 0, 2))
            for e in ex
        ])
        wgu = np.stack([
            np.stack([eg[e], eu[e]]).reshape(2, H // 128, 128, 2, I // 2)
            .transpose(3, 2, 0, 1, 4)
            for e in ex
        ])
        wdp = np.stack([
            (ed[e].astype(np.float32) * 64.0).astype(ml_dtypes.float8_e4m3)
            .reshape(2, 2, 128, H).transpose(2, 0, 1, 3) for e in ex
        ])
        wGc = np.stack([
            comb[gidx_all[e], e].astype(np.float32) for e in ex
        ]) / 512.0  # fold out the 8x wu and 64x wd fp8 scales
        for k, e in enumerate(ex):
            wGc[k, int(counts[e]):] = 0.0
        in_maps.append({
            "xT16": xT4,
            "xgT16": xgT,
            "wgu16": np.ascontiguousarray(wgu),
            "wd16": np.ascontiguousarray(wdp),
            "sgsu16": np.ascontiguousarray(
                np.concatenate([sg[:, c * SIC:(c + 1) * SIC],
                                su[:, c * SIC:(c + 1) * SIC]], axis=1)
                .reshape(H // 128, 128, 2 * SIC).transpose(1, 0, 2)),
            "sd16": np.ascontiguousarray(sd[c * SIC:(c + 1) * SIC, :]),
            "sidx": np.ascontiguousarray(
                sidx_all[ex].reshape(EPC * NCC, 128).T),
            "wG": np.ascontiguousarray(wGc.reshape(EPC * NCC, 128).T),
        })

    # batch token ranges + chunk-level collision edges (union across cores —
    # SPMD shares one program)
    brange = {}
    edge_set = set()
    for c in range(N_CORES):
        rng = {}
        for k in range(EPC):
            e = EPC * c + k
            for i in range(NCC):
                r = sidx_all[e, i * 128:(i + 1) * 128]
                r = r[r < OOB]
                if len(r):
                    rng[(k, i)] = (int(r.min()), int(r.max()))
        for i in range(NCC):
            for jj in range(NCC):
                a = rng.get((0, i))
                b = rng.get((1, jj))
                if a and b and a[0] <= b[1] and b[0] <= a[1]:
                    edge_set.add((i, jj))
        for bi, (k, c0, c1) in enumerate(batches):
            e = EPC * c + k
            r = sidx_all[e, c0 * 128:(c1 + 1) * 128]
            r = r[r < OOB]
            if len(r):
                lo, hi = int(r.min()), int(r.max())
                old = brange.get(bi)
                brange[bi] = (min(old[0], lo), max(old[1], hi)) if old else (lo, hi)
    edges = tuple(sorted(edge_set))
    touch_tq = {}
    touch_q = {}
    for bi, (lo, hi) in brange.items():
        touch_tq[bi] = tuple(range(lo // QT, hi // QT + 1))
        for h in range(lo // TH, hi // TH + 1):
            touch_q.setdefault(h, set()).add(bi)
    touch_tq_t = tuple(sorted(touch_tq.items()))
    touch_q_t = tuple(sorted((h, tuple(sorted(v))) for h, v in touch_q.items()))

    key = (C_use, C_pad, batches, edges, touch_tq_t, touch_q_t, NHALF)
    if key not in _nc_cache:
        _nc_cache[key] = _build(
            C_use, C_pad, batches, edges,
            dict(touch_tq_t), {h: v for h, v in touch_q_t})
    nc = _nc_cache[key]
    trace = bool(int(os.environ.get("KERNEL_TRACE", "0")))
    res = run_bass_kernel_spmd(
        nc, in_maps, core_ids=list(range(N_CORES)), trace=trace
    )
    last_exec_time_ns = res.exec_time_ns

    # reassemble: RS chunk h gives core c rows [h*TH + c*rows : +rows]
    rows = TH // N_CORES
    out = np.empty((T, Hh), np.float32)
    for c in range(N_CORES):
        yo = res.results[c]["y_out"]
        for h in range(NHALF):
            out[h * TH + c * rows:h * TH + (c + 1) * rows] = yo[h * rows:(h + 1) * rows]
    return out.reshape(B, S, Hh).astype(np.float32)



# revision 2
# speedup vs baseline: 1.2261x; 1.2261x over previous
"""DeepseekV3 MoE layer on 8 Trainium2 NeuronCores.

Strategy (expert-parallel, per sharding hint):
- Each core owns 2 of the 16 routed experts. The host routes tokens by top-4
  gate scores (fp32, identical to reference) and ships each core its experts'
  gathered tokens pre-transposed, plus the normalized combine weights
  (host-side gate math, same class of work as the top-k routing).
- All device inputs are shipped in their exact SBUF layouts so every preload
  DMA is contiguous per partition (8-16KB lines) and balanced across the
  three DMA queues (sync / scalar / gpsimd).
- The device runs the SwiGLU expert MLPs fp16 (fp32 PSUM), scales outputs by
  the combine weights into per-expert staging buffers, and scatter-adds them
  into per-WINDOW partial-output tensors y_acc[h] in DRAM (NW windows of
  T/NW tokens). A chunk whose tokens span two windows is scattered once per
  window with host-rebased indices (rows outside the window are OOB-masked
  and skipped by the DMA engine). Per-window tensors keep the shadow-memory
  dependency tracker from serializing later scatters behind earlier
  reduce-scatters.
- The shared expert is sharded along its intermediate dim (128 of 1024 per
  core), computed weight-stationary so its intermediate lands pre-transposed
  ([i, t]); its dense per-window output initializes y_acc[h].
- Each window's ReduceScatter fires as soon as that window's scatters land,
  overlapping the collective with the remaining compute. A tiny warm-up
  collective at kernel start absorbs the first-collective ramp cost.
- The host reassembles the fp16 outputs and casts (pure unshard).
"""

import os
import sys
import types

sys.path.insert(0, "/opt/trn_rl_repo")

# antenv.axon_hooks shim so trace=True works under axon (profiling only).
if "antenv.axon_hooks" not in sys.modules:
    _hook_holder = [None]
    _hooks_mod = types.ModuleType("antenv.axon_hooks")
    _hooks_mod.set_axon_ntff_profile_hook = lambda h: _hook_holder.__setitem__(0, h)
    _hooks_mod.get_axon_ntff_profile_hook = lambda: _hook_holder[0]
    sys.modules["antenv.axon_hooks"] = _hooks_mod
    try:
        from trn_agent_boot.trn_boot import _ntff_profile_via_ctypes

        _hook_holder[0] = _ntff_profile_via_ctypes("/opt/axon/libaxon_pjrt.so")
    except Exception:
        pass

import ml_dtypes
import numpy as np

import concourse.bass as bass
import concourse.mybir as mybir
from concourse import bacc
from concourse.tile import TileContext, add_dep_helper
from concourse.bass_utils import run_bass_kernel_spmd

N_CORES = 8
T, H, E, I = 2048, 1024, 16, 512
TOPK = 4
SIC = 128  # shared-expert intermediate slice per core (1024 / 8)
EPC = 2  # experts per core
OOB = 1 << 20
NW = int(os.environ.get('KERNEL_NHALF', '4'))  # reduce-scatter windows
W = T // NW  # window row count (also the dense-write granularity)

F16 = mybir.dt.float16
F8 = mybir.dt.float8e4
F32 = mybir.dt.float32
I32 = mybir.dt.int32
AF = mybir.ActivationFunctionType

_nc_cache = {}
last_exec_time_ns = None


def _build(C_use, C_pad, scols, coll):
    """scols[h]: tuple of (e, cc) chunks scattered into window h, in
    emission order. coll[h]: tuple of (i, j) pairs — within window h,
    scatter j must wait for scatter i (cross-expert RMW collisions)."""
    NCC = C_pad // 128
    WT = W // 128  # 128-row tiles per window
    nc = bacc.Bacc(trn_type="TRN2", target_bir_lowering=False, num_devices=N_CORES)

    NS = sum(len(s) for s in scols)

    # ---- I/O (all pre-arranged to SBUF layout on host; contiguous DMAs) ----
    xT16 = nc.dram_tensor("xT16", [NW, 128, H // 128, W], F16, kind="ExternalInput")
    xgT16 = nc.dram_tensor("xgT16", [EPC, 128, H // 128, C_pad], F16, kind="ExternalInput")
    wgu16 = nc.dram_tensor("wgu16", [EPC, 2, 128, 2, H // 128, I // 2], F16, kind="ExternalInput")
    wd16 = nc.dram_tensor("wd16", [EPC, 128, 2, 2, H], F8, kind="ExternalInput")
    sgsu16 = nc.dram_tensor("sgsu16", [128, H // 128, 2 * SIC], F16, kind="ExternalInput")
    sd16 = nc.dram_tensor("sd16", [SIC, H], F16, kind="ExternalInput")
    sidx = nc.dram_tensor("sidx", [128, NS], I32, kind="ExternalInput")
    wG = nc.dram_tensor("wG", [128, EPC * NCC], F32, kind="ExternalInput")

    y_acc = [nc.dram_tensor(f"y_acc{h}", [W, H], F16) for h in range(NW)]
    rows = W // N_CORES
    rs_b = nc.dram_tensor("rs_b", [NW * rows, H], F16)
    y_out = nc.dram_tensor("y_out", [NW * rows, H], F16, kind="ExternalOutput")
    warm_i = nc.dram_tensor("warm_i", [64, 128], F32)
    warm_o = nc.dram_tensor("warm_o", [8, 128], F32)

    SS = 2 * SIC  # 256

    with TileContext(nc) as tc:
        with (
            tc.tile_pool(name="res", bufs=1) as res,
            tc.tile_pool(name="sc", bufs=4) as scp,
            tc.tile_pool(name="ds", bufs=2) as dsp,
            tc.tile_pool(name="ps_a", bufs=4, space="PSUM") as ps_a,
            tc.tile_pool(name="ps_gu", bufs=2, space="PSUM") as ps_gu,
        ):
            # warm-up collective: absorbs first-collective ramp during preload
            nc.gpsimd.collective_compute(
                "ReduceScatter",
                mybir.AluOpType.add,
                replica_groups=[list(range(N_CORES))],
                ins=[warm_i.ap().opt()],
                outs=[warm_o.ap().opt()],
            )

            # ---- resident tiles ----
            xT_sb = [res.tile([128, H // 128, W], F16, tag=f"xT{q}",
                              name=f"xT_sb{q}") for q in range(NW)]
            xgT_sb = res.tile([128, EPC, H // 128, C_pad], F16, tag="xgT")
            wgu_sb = res.tile([128, EPC, 2, 2, H // 128, I // 2], F16, tag="wgu")
            wd_sb = res.tile([128, EPC, 2, 2, H], F8, tag="wd")
            sgsu_sb = res.tile([128, H // 128, SS], F16, tag="sgsu")
            sd_sb = res.tile([128, H], F16, tag="sd")
            sidx_sb = res.tile([128, NS], I32, tag="sidx")
            wG_sb = res.tile([128, EPC * NCC], F32, tag="wG")
            p_sb = res.tile([128, EPC, 2, 2, C_pad], F8, tag="p")
            sp_sb = res.tile([128, T], F16, tag="sp")
            yg_sb = [res.tile([128, NCC, H], F16, tag=f"yg{e}",
                              name=f"yg_sb{e}") for e in range(EPC)]

            # ---- preload: balanced across the three DMA queues. ----
            nc.sync.dma_start(xT_sb[0][:], xT16.ap()[0])
            nc.sync.dma_start(sd_sb[:], sd16.ap())
            for q in range(1, NW):
                nc.sync.dma_start(xT_sb[q][:], xT16.ap()[q])
            nc.sync.dma_start(sidx_sb[:], sidx.ap())
            nc.sync.dma_start(wG_sb[:], wG.ap())
            nc.scalar.dma_start(sgsu_sb[:], sgsu16.ap())
            nc.scalar.dma_start(wgu_sb[:, 0, 0], wgu16.ap()[0, 0])
            nc.scalar.dma_start(wgu_sb[:, 0, 1], wgu16.ap()[0, 1])
            nc.scalar.dma_start(wd_sb[:, 0], wd16.ap()[0])
            nc.scalar.dma_start(wd_sb[:, 1], wd16.ap()[1])
            nc.gpsimd.dma_start(xgT_sb[:, 0], xgT16.ap()[0])
            nc.gpsimd.dma_start(wgu_sb[:, 1, 0], wgu16.ap()[1, 0])
            nc.gpsimd.dma_start(wgu_sb[:, 1, 1], wgu16.ap()[1, 1])
            nc.gpsimd.dma_start(xgT_sb[:, 1], xgT16.ap()[1])

            # zero the pad columns of p (read by down-matmul lhsT chunks)
            if C_pad > C_use:
                nc.vector.memset(p_sb[:, :, :, :, C_use:C_pad], 0)

            # gate/up token blocks of 256
            segs = []
            s0 = 0
            while s0 < C_use:
                s1 = min(s0 + 256, C_use)
                segs.append((s0, s1))
                s0 = s1

            # ---- shared expert: weight-stationary gate/up; sp lands
            # pre-transposed [i, t] so the down matmul needs no transposes ----
            def emit_su(h):
                ps_ic = []
                for ic in range(2):
                    psu = ps_a.tile([128, W], F32, tag="psa")
                    for ho in range(H // 128):
                        nc.tensor.matmul(
                            psu[:],
                            lhsT=sgsu_sb[:, ho, ic * 128:(ic + 1) * 128],
                            rhs=xT_sb[h][:, ho, :],
                            start=(ho == 0),
                            stop=(ho == H // 128 - 1),
                        )
                    ps_ic.append(psu)
                sg_t = scp.tile([128, W], F16, tag="sg")
                nc.scalar.activation(sg_t[:], ps_ic[0][:], AF.Silu)
                nc.vector.tensor_tensor(
                    out=sp_sb[:, h * W:(h + 1) * W], in0=sg_t[:], in1=ps_ic[1][:],
                    op=mybir.AluOpType.mult,
                )

            dense_wr = [None] * NW
            su_done = [False] * NW

            def emit_dense(h):
                ys = dsp.tile([128, WT, H], F16, tag="ys")
                for tc4 in range(WT):
                    t0 = h * W + tc4 * 128
                    for hf in range(2):
                        pso = ps_a.tile([128, 512], F32, tag="psa")
                        nc.tensor.matmul(
                            pso[:],
                            lhsT=sp_sb[:, t0:t0 + 128],
                            rhs=sd_sb[:, hf * 512:(hf + 1) * 512],
                            start=True,
                            stop=True,
                        )
                        nc.scalar.activation(
                            ys[:, tc4, hf * 512:(hf + 1) * 512], pso[:], AF.Copy)
                dense_wr[h] = nc.sync.dma_start(
                    y_acc[h].ap().rearrange("(tc p) h -> p tc h", p=128),
                    ys[:],
                )

            def ensure_dense(h):
                if dense_wr[h] is None:
                    if not su_done[h]:
                        emit_su(h)
                        su_done[h] = True
                    emit_dense(h)

            # ---- routed experts: g/u -> p = silu(g)*u for one token block ----
            def emit_gu(e, a, b):
                for it in range(I // 128):
                    pg_full = ps_gu.tile([128, 512], F32, tag="pg")
                    pg = pg_full[:, :b - a]
                    pu_full = ps_gu.tile([128, 512], F32, tag="pu")
                    pu = pu_full[:, :b - a]
                    for ho in range(H // 128):
                        nc.tensor.matmul(
                            pg[:],
                            lhsT=wgu_sb[:, e, it // 2, 0, ho,
                                        (it % 2) * 128:(it % 2) * 128 + 128],
                            rhs=xgT_sb[:, e, ho, a:b],
                            start=(ho == 0),
                            stop=(ho == H // 128 - 1),
                        )
                        nc.tensor.matmul(
                            pu[:],
                            lhsT=wgu_sb[:, e, it // 2, 1, ho,
                                        (it % 2) * 128:(it % 2) * 128 + 128],
                            rhs=xgT_sb[:, e, ho, a:b],
                            start=(ho == 0),
                            stop=(ho == H // 128 - 1),
                        )
                    sg2_full = scp.tile([128, 512], F16, tag="sg2")
                    sg2 = sg2_full[:, :b - a]
                    nc.scalar.activation(sg2[:], pg[:], AF.Silu)
                    nc.vector.tensor_tensor(
                        out=p_sb[:, e, it // 2, it % 2, a:b], in0=sg2[:],
                        in1=pu[:], op=mybir.AluOpType.mult,
                    )

            gu_blocks = [0, 0]

            def gu_through(e, cc):
                while gu_blocks[e] * 256 < (cc + 1) * 128:
                    a, b = segs[gu_blocks[e]]
                    emit_gu(e, a, b)
                    gu_blocks[e] += 1

            # ---- routed expert down matmul + combine-weight scale ----
            down_done = {}

            def emit_down(e, cc):
                if (e, cc) in down_done:
                    return
                j = e * NCC + cc
                for hf in range(2):
                    py = ps_a.tile([128, 512], F32, tag="psa")
                    for kp in range(2):
                        nc.tensor.matmul(
                            py[:],
                            lhsT=p_sb[:, e, kp, :, cc * 128:(cc + 1) * 128],
                            rhs=wd_sb[:, e, kp, :, hf * 512:(hf + 1) * 512],
                            start=(kp == 0),
                            stop=(kp == 1),
                            perf_mode=mybir.MatmulPerfMode.DoubleRow,
                        )
                    nc.vector.tensor_scalar_mul(
                        yg_sb[e][:, cc, hf * 512:(hf + 1) * 512],
                        py[:], wG_sb[:, j:j + 1])
                down_done[(e, cc)] = True

            # ---- per-window: downs -> scatters -> eager ReduceScatter ----
            ensure_dense(0)
            ensure_dense(1)
            col_j = 0
            for h in range(NW):
                ensure_dense(h)
                win_sc = []
                for idx, (e, cc) in enumerate(scols[h]):
                    gu_through(e, cc)
                    emit_down(e, cc)
                    sc = nc.gpsimd.indirect_dma_start(
                        out=y_acc[h][:],
                        out_offset=bass.IndirectOffsetOnAxis(
                            ap=sidx_sb[:, col_j:col_j + 1], axis=0),
                        in_=yg_sb[e][:, cc, :],
                        in_offset=None,
                        bounds_check=W - 1,
                        oob_is_err=False,
                        compute_op=mybir.AluOpType.add,
                    )
                    col_j += 1
                    add_dep_helper(sc.ins, dense_wr[h].ins,
                                   reason="scatter after dense init")
                    for (i, jdx) in coll[h]:
                        if jdx == idx:
                            add_dep_helper(sc.ins, win_sc[i].ins,
                                           reason="serialize colliding scatters")
                    win_sc.append(sc)
                cc_inst = nc.gpsimd.collective_compute(
                    "ReduceScatter",
                    mybir.AluOpType.add,
                    replica_groups=[list(range(N_CORES))],
                    ins=[y_acc[h].ap().opt()],
                    outs=[rs_b.ap()[h * rows:(h + 1) * rows, :].opt()],
                )
                for sc in win_sc:
                    add_dep_helper(cc_inst.ins, sc.ins, reason="rs after scatters")
                add_dep_helper(cc_inst.ins, dense_wr[h].ins,
                               reason="rs after dense init")
                out_wr = nc.sync.dma_start(
                    y_out.ap()[h * rows:(h + 1) * rows, :],
                    rs_b.ap()[h * rows:(h + 1) * rows, :],
                )
                add_dep_helper(out_wr.ins, cc_inst.ins, reason="copy rs out")
                if h + 2 < NW:
                    ensure_dense(h + 2)

    nc.compile()
    return nc


def kernel(hidden_states, gate_w, expert_gate, expert_up, expert_down,
           shared_gate, shared_up, shared_down):
    global last_exec_time_ns
    B, S, Hh = hidden_states.shape
    x = np.asarray(hidden_states, np.float32).reshape(-1, Hh)

    # ---- host-side routing (the all-to-all dispatch, done as sharding) ----
    gw = np.asarray(gate_w, np.float32)
    scores = 1.0 / (1.0 + np.exp(-(x @ gw.T)))
    order = np.argsort(-scores, axis=1, kind="stable")[:, :TOPK]
    topk_w = np.take_along_axis(scores, order, axis=1)
    topk_w = topk_w / (topk_w.sum(-1, keepdims=True) + 1e-20)
    comb = np.zeros((T, E), np.float32)
    np.add.at(comb, (np.arange(T)[:, None], order), topk_w)

    sel = np.zeros((T, E), dtype=bool)
    sel[np.arange(T)[:, None], order] = True
    counts = sel.sum(0)
    C_use = int(max(64, -(-int(counts.max()) // 64) * 64))
    C_use = min(C_use, T)
    C_pad = -(-C_use // 128) * 128
    NCC = C_pad // 128

    gidx_all = np.zeros((E, C_pad), np.int32)
    sidx_all = np.full((E, C_pad), OOB, np.int32)
    for e in range(E):
        lst = np.nonzero(sel[:, e])[0].astype(np.int32)
        gidx_all[e, :len(lst)] = lst
        sidx_all[e, :len(lst)] = lst

    # ---- per-window scatter columns (union across cores; SPMD shares one
    # program). Chunk (e, cc) scattered into window h iff any core has a
    # token of local expert e, chunk cc inside window h. ----
    scols = []
    coll = []
    for h in range(NW):
        wcols = []
        for cc in range(NCC):
            for k in range(EPC):
                hit = False
                for c in range(N_CORES):
                    r = sidx_all[EPC * c + k, cc * 128:(cc + 1) * 128]
                    if np.any((r >= h * W) & (r < (h + 1) * W)):
                        hit = True
                        break
                if hit:
                    wcols.append((k, cc))
        # cross-expert RMW collisions within the window (any core)
        wdeps = []
        for jdx in range(len(wcols)):
            for i in range(jdx):
                ke_i, cc_i = wcols[i]
                ke_j, cc_j = wcols[jdx]
                if ke_i == ke_j:
                    continue
                hit = False
                for c in range(N_CORES):
                    ri = sidx_all[EPC * c + ke_i, cc_i * 128:(cc_i + 1) * 128]
                    rj = sidx_all[EPC * c + ke_j, cc_j * 128:(cc_j + 1) * 128]
                    ri = ri[(ri >= h * W) & (ri < (h + 1) * W)]
                    rj = rj[(rj >= h * W) & (rj < (h + 1) * W)]
                    if len(ri) and len(rj) and len(np.intersect1d(ri, rj)):
                        hit = True
                        break
                if hit:
                    wdeps.append((i, jdx))
        scols.append(tuple(wcols))
        coll.append(tuple(wdeps))
    scols = tuple(scols)
    coll = tuple(coll)

    # ---- cast / pack per-core inputs in exact SBUF layouts ----
    x16 = x.astype(np.float16)
    xT4 = np.ascontiguousarray(
        x16.T.reshape(H // 128, 128, NW, W).transpose(2, 1, 0, 3))
    eg = np.asarray(expert_gate, np.float32).astype(np.float16)
    eu = (np.asarray(expert_up, np.float32) * 8.0).astype(np.float16)
    ed = np.asarray(expert_down, np.float32)
    sg = np.asarray(shared_gate, np.float32).astype(np.float16)
    su = np.asarray(shared_up, np.float32).astype(np.float16)
    sd = np.asarray(shared_down, np.float32).astype(np.float16)

    in_maps = []
    for c in range(N_CORES):
        ex = [EPC * c + k for k in range(EPC)]
        xgT = np.stack([
            np.ascontiguousarray(
                x16[gidx_all[e]].T.reshape(H // 128, 128, C_pad).transpose(1, 0, 2))
            for e in ex
        ])
        wgu = np.stack([
            np.stack([eg[e], eu[e]]).reshape(2, H // 128, 128, 2, I // 2)
            .transpose(3, 2, 0, 1, 4)
            for e in ex
        ])
        wdp = np.stack([
            (ed[e].astype(np.float32) * 64.0).astype(ml_dtypes.float8_e4m3)
            .reshape(2, 2, 128, H).transpose(2, 0, 1, 3) for e in ex
        ])
        wGc = np.stack([
            comb[gidx_all[e], e].astype(np.float32) for e in ex
        ]) / 512.0  # fold out the 8x wu and 64x wd fp8 scales
        for k, e in enumerate(ex):
            wGc[k, int(counts[e]):] = 0.0
        # per-window rebased scatter indices (OOB outside the window)
        sidx_cols = []
        for h in range(NW):
            for (k, cc) in scols[h]:
                r = sidx_all[ex[k], cc * 128:(cc + 1) * 128]
                inw = (r >= h * W) & (r < (h + 1) * W)
                sidx_cols.append(np.where(inw, r - h * W, OOB).astype(np.int32))
        in_maps.append({
            "xT16": xT4,
            "xgT16": xgT,
            "wgu16": np.ascontiguousarray(wgu),
            "wd16": np.ascontiguousarray(wdp),
            "sgsu16": np.ascontiguousarray(
                np.concatenate([sg[:, c * SIC:(c + 1) * SIC],
                                su[:, c * SIC:(c + 1) * SIC]], axis=1)
                .reshape(H // 128, 128, 2 * SIC).transpose(1, 0, 2)),
            "sd16": np.ascontiguousarray(sd[c * SIC:(c + 1) * SIC, :]),
            "sidx": np.ascontiguousarray(np.stack(sidx_cols, axis=1)),
            "wG": np.ascontiguousarray(wGc.reshape(EPC * NCC, 128).T),
        })

    key = (C_use, C_pad, scols, coll, NW)
    if key not in _nc_cache:
        _nc_cache[key] = _build(C_use, C_pad, scols, coll)
    nc = _nc_cache[key]
    trace = bool(int(os.environ.get("KERNEL_TRACE", "0")))
    res = run_bass_kernel_spmd(
        nc, in_maps, core_ids=list(range(N_CORES)), trace=trace
    )
    last_exec_time_ns = res.exec_time_ns

    # reassemble: RS window h gives core c rows [h*W + c*rows : +rows]
    rows = W // N_CORES
    out = np.empty((T, Hh), np.float32)
    for c in range(N_CORES):
        yo = res.results[c]["y_out"]
        for h in range(NW):
            out[h * W + c * rows:h * W + (c + 1) * rows] = yo[h * rows:(h + 1) * rows]
    return out.reshape(B, S, Hh).astype(np.float32)
